# revision 36
# baseline (speedup 1.0000x reference)
"""Llama-style transformer block on 8 TRN2 NeuronCores.

Megatron tensor-parallel with feature-major (transposed) activations,
v6: fully software-pipelined emission stream, collective-free RMSNorm.

  - Residual stream TRANSPOSED (x^T: [D, S]); every matmul contracts over
    the partition dim with zero on-chip transposes.
  - Per core: 4 heads (512 of 4096 q/k/v dims), 1376->1408 FFN dims.
  - RMSNorm WITHOUT AllReduce: every core receives the full activation
    through the z / h AllGathers anyway, so the sum-of-squares is
    computed locally from the gathered tiles (32 ones-matmuls per chunk)
    -- no cross-core latency chain ever touches the Tensor queue.
  - Pipeline: QKV(c) -> attn(c) -> wo(c) -> RS(y_c) -> residual(c) ->
    AllGather(h_c); FFN chunks follow, each ReduceScatter overlapping the
    next chunk's matmuls; last FFN chunk split in token-halves to shrink
    the exposed tail RS.
  - Attention: two heads processed in lockstep so the score->exp->AV
    ping-pong of one head fills the other's ScalarE latency; softmax
    denominators accumulated on DVE, one ones-matmul per (chunk, head);
    normalization deferred to the attnT write.
  - norm scales: ScalarE sqrt + DVE reciprocal_approx_fast + GpSimd
    partition broadcast (never on the Tensor queue).
  - q/k stay in SBUF (no DRAM bounce); h kept in bf16, residual re-added
    in f32 at the end.
  - DMA rings: sync = prompt weight/activation streams, scalar = second
    weight stream + residual stores, gpsimd = collective triggers and
    collective-gated loads.
"""

import math
from contextlib import ExitStack

import ml_dtypes
import numpy as np

import concourse.bass as bass
import concourse.mybir as mybir
import concourse.tile as tile
from concourse import bacc
from concourse.bass_utils import run_bass_kernel_spmd

S = 2048
D = 4096
HD = 128
NH = 32
F = 11008
CORES = 8
NHC = NH // CORES          # heads per core = 4
DQ = NHC * HD              # q/k/v dims per core = 512
FC = F // CORES            # ffn dims per core = 1376
FT = 11                    # padded f-tiles per core
FP = FT * 128
EPS = 1e-5
P = 128
NCH = 4                    # 512-token chunks
CW = S // NCH              # chunk width = 512
DT = D // P                # d tiles = 32
ST = S // P                # s tiles = 16
HW = CW // 2               # half chunk = 256 (tail RS split)

CDT = mybir.dt.bfloat16
NP_CDT = ml_dtypes.bfloat16

_COMPILED = None

# o-chunks: 3 full 512-token chunks + 2 half chunks for the tail
OCH = [(0, 0, CW), (1, CW, CW), (2, 2 * CW, CW),
       (3, 3 * CW, HW), (4, 3 * CW + HW, HW)]


def _build():
    nc = bacc.Bacc("TRN2", target_bir_lowering=False, debug=False,
                   num_devices=CORES)
    f32 = mybir.dt.float32

    # ---- kernel I/O ----
    xT_bf = nc.declare_dram_parameter("xT_bf", [DQ, S], CDT, isOutput=False)
    w_qk = nc.declare_dram_parameter("w_qk", [8, P, DT, P], CDT, isOutput=False)
    w_v = nc.declare_dram_parameter("w_v", [DT, P, DQ], CDT, isOutput=False)
    w_o = nc.declare_dram_parameter("w_o", [P, 32, 4, P], CDT, isOutput=False)
    w_1 = nc.declare_dram_parameter("w_1", [FT, P, DT, P], CDT, isOutput=False)
    w_3 = nc.declare_dram_parameter("w_3", [FT, P, DT, P], CDT, isOutput=False)
    w_2 = nc.declare_dram_parameter("w_2", [32, P, FT, P], CDT, isOutput=False)
    cos2 = nc.declare_dram_parameter("cos2", [P, S], CDT, isOutput=False)
    sinsg2 = nc.declare_dram_parameter("sinsg2", [P, S], CDT, isOutput=False)
    dmask = nc.declare_dram_parameter("dmask", [P, P], f32, isOutput=False)
    outT_s = nc.declare_dram_parameter("outT_s", [DQ, S], f32, isOutput=True)

    # ---- internal DRAM ----
    s_scr = [nc.dram_tensor(f"s_scr{c}", [1, CW], f32) for c in range(NCH)]
    zs_cc = [nc.dram_tensor(f"zs_cc{c}", [DQ, CW], CDT) for c in range(NCH)]
    zT_ag = [nc.dram_tensor(f"zT_ag{c}", [D, CW], CDT, addr_space="Shared")
             for c in range(NCH)]
    yT_cc = [nc.dram_tensor(f"yT_cc{c}", [D, CW], CDT) for c in range(NCH)]
    y_rs = [nc.dram_tensor(f"y_rs{c}", [DQ, CW], CDT) for c in range(NCH)]
    hn_cc = [nc.dram_tensor(f"hn_cc{c}", [DQ, CW], CDT) for c in range(NCH)]
    hnT_ag = [nc.dram_tensor(f"hnT_ag{c}", [D, CW], CDT, addr_space="Shared")
              for c in range(NCH)]
    oT_cc = [nc.dram_tensor(f"oT_cc{k}", [D, w], CDT) for k, _, w in OCH]
    o_rs = [nc.dram_tensor(f"o_rs{k}", [DQ, w], CDT) for k, _, w in OCH]

    RG = [list(range(CORES))]
    ADD = mybir.AluOpType.add
    BYP = mybir.AluOpType.bypass
    EXP = mybir.ActivationFunctionType.Exp
    SQRT = mybir.ActivationFunctionType.Sqrt
    SILU = mybir.ActivationFunctionType.Silu
    ISQ = 1.0 / math.sqrt(HD)

    def ch(c):
        return slice(CW * c, CW * (c + 1))

    with tile.TileContext(nc) as tc:
        es = ExitStack()
        persist = es.enter_context(tc.tile_pool(name="persist", bufs=1))
        ps_qk = es.enter_context(tc.tile_pool(name="ps_qk", bufs=2,
                                              space="PSUM"))
        ps_v = es.enter_context(tc.tile_pool(name="ps_v", bufs=2,
                                             space="PSUM"))
        ps_sc = es.enter_context(tc.tile_pool(name="ps_sc", bufs=2,
                                              space="PSUM"))
        ps_av = es.enter_context(tc.tile_pool(name="ps_av", bufs=2,
                                              space="PSUM"))

        # ---------- persistent SBUF ----------
        ones = persist.tile([P, 1], CDT)
        nc.vector.memset(ones[:], 1.0)
        ones_f = persist.tile([P, 1], f32)
        nc.vector.memset(ones_f[:], 1.0)
        eps_sb = persist.tile([P, 1], f32)
        nc.vector.memset(eps_sb[:], EPS)
        dmask_sb = persist.tile([P, P], f32)
        nc.sync.dma_start(out=dmask_sb[:], in_=dmask[:])
        cs_sb = persist.tile([P, S], CDT)
        sn_sb = persist.tile([P, S], CDT)
        cos_raw = persist.tile([P, S], CDT)
        sin_raw = persist.tile([P, S], CDT)
        nc.sync.dma_start(out=cos_raw[:], in_=cos2[:])
        nc.sync.dma_start(out=sin_raw[:], in_=sinsg2[:])
        s_tok = persist.tile([P, ST], f32)
        s2rep = persist.tile([P, S], CDT)
        kts = [persist.tile([P, S], CDT, tag=f"kts{h}", name=f"kts{h}")
               for h in range(NHC)]
        v_sb = persist.tile([P, ST, DQ], CDT)
        hT_bf = [persist.tile([P, S], CDT, tag=f"hT{i}", name=f"hT{i}")
                 for i in range(4)]

        # ---------- t=0: AG(z) triggers ASAP ----------
        nc.sync.dma_start(out=zs_cc[0][:], in_=xT_bf[:, ch(0)])
        nc.gpsimd.collective_compute(
            "AllGather", BYP, ins=[zs_cc[0][:]], outs=[zT_ag[0][:]],
            replica_groups=RG)
        for c in range(1, NCH):
            nc.sync.dma_start(out=zs_cc[c][:], in_=xT_bf[:, ch(c)])
            nc.gpsimd.collective_compute(
                "AllGather", BYP, ins=[zs_cc[c][:]], outs=[zT_ag[c][:]],
                replica_groups=RG)

        # ---------- pools ----------
        xin_cm = tc.tile_pool(name="xinp", bufs=1)
        xinp = xin_cm.__enter__()
        wk_cm = tc.tile_pool(name="wkp", bufs=4)
        wkp = wk_cm.__enter__()
        wv_cm = tc.tile_pool(name="wvp", bufs=2)
        wvp = wv_cm.__enter__()
        wo_cm = tc.tile_pool(name="wop", bufs=4)
        wop = wo_cm.__enter__()
        rope_cm = tc.tile_pool(name="ropep", bufs=2)
        ropep = rope_cm.__enter__()
        q_cm = tc.tile_pool(name="qp", bufs=1)
        qp = q_cm.__enter__()
        sq_cm = tc.tile_pool(name="sqp", bufs=2)
        sqp = sq_cm.__enter__()
        at_cm = tc.tile_pool(name="atp", bufs=1)
        atp = at_cm.__enter__()
        exp_cm = tc.tile_pool(name="expp", bufs=4)
        expp = exp_cm.__enter__()
        esum_cm = tc.tile_pool(name="esump", bufs=2)
        esump = esum_cm.__enter__()
        rec_cm = tc.tile_pool(name="recp", bufs=2)
        recp = rec_cm.__enter__()
        rep_cm = tc.tile_pool(name="repp", bufs=2)
        repp = rep_cm.__enter__()
        yt_cm = tc.tile_pool(name="ytp", bufs=2)
        ytp = yt_cm.__enter__()
        g_cm = tc.tile_pool(name="gp", bufs=1)
        gp = g_cm.__enter__()
        w2_cm = tc.tile_pool(name="w2p", bufs=3)
        w2p = w2_cm.__enter__()
        fs_cm = tc.tile_pool(name="fsp", bufs=2)
        fsp = fs_cm.__enter__()
        og_cm = tc.tile_pool(name="ogp", bufs=2)
        ogp = og_cm.__enter__()
        resid_cm = tc.tile_pool(name="resid", bufs=2)
        resid = resid_cm.__enter__()

        def rms_scale(pt_row, dst_rep):
            """PSUM row [1,CW] of sum(x^2) -> rsqrt chain -> broadcast to
            dst_rep [P,CW].  Never touches the Tensor queue."""
            rc = recp.tile([1, CW], f32, tag="rc")
            nc.scalar.activation(out=rc[:], in_=pt_row, func=SQRT,
                                 bias=eps_sb[0:1], scale=1.0 / D)
            nc.vector.reciprocal_approx_fast(out=rc[:], in_=rc[:])
            nc.gpsimd.partition_broadcast(dst_rep, rc[:])
            return rc

        def qkv_chunk(c, q_t):
            """stats + V + Q/K (+RoPE) for chunk c from gathered z."""
            with nc.named_scope(f"qkv_c{c}"):
                zt = xinp.tile([P, DT, CW], CDT, tag="xin", name="zt")
                zv = zT_ag[c][:].rearrange("(kt p) s -> p kt s", p=P)
                for q4 in range(4):
                    nc.sync.dma_start(out=zt[:, 8 * q4:8 * (q4 + 1), :],
                                      in_=zv[:, 8 * q4:8 * (q4 + 1), :])
                # local stats from gathered z: ssq = sum_d z^2
                spt = ps_sc.tile([P, CW], f32, tag="scp")
                for kt in range(DT):
                    sq = sqp.tile([P, CW], CDT, tag="sq")
                    nc.vector.tensor_mul(sq[:], zt[:, kt, :], zt[:, kt, :])
                    nc.tensor.matmul(spt[0:1, :], ones[:], sq[:],
                                     start=(kt == 0), stop=(kt == DT - 1))
                s1rep = repp.tile([P, CW], f32, tag="rrep", name="s1rep")
                rc = rms_scale(spt[0:1, :], s1rep[:])
                # fold s1 into this chunk's rope tables
                nc.vector.tensor_mul(cs_sb[:, ch(c)], cos_raw[:, ch(c)],
                                     s1rep[:])
                nc.vector.tensor_mul(sn_sb[:, ch(c)], sin_raw[:, ch(c)],
                                     s1rep[:])
                # token-major copy of s1 for the V scale (via DRAM bounce)
                nc.gpsimd.dma_start(out=s_scr[c][:], in_=rc[:])
                nc.sync.dma_start(
                    out=s_tok[:, 4 * c:4 * (c + 1)],
                    in_=s_scr[c][:].rearrange("o (j p) -> p (o j)", p=P))
                # V: single pass over kt, 4 PSUM banks (ps_v x2 + ps_qk x2)
                pts = [ps_v.tile([P, DQ], f32, tag="pv", name=f"pv{jj}")
                       for jj in range(2)]
                pts += [ps_qk.tile([P, DQ], f32, tag="pqk", name=f"pq{jj}")
                        for jj in range(2)]
                for kt in range(DT):
                    wv_t = wvp.tile([P, DQ], CDT, tag="wv")
                    nc.scalar.dma_start(out=wv_t[:], in_=w_v[kt])
                    for j in range(4):
                        tok = slice(P * j, P * (j + 1))
                        nc.tensor.matmul(
                            pts[j][:], zt[:, kt, tok], wv_t[:],
                            start=(kt == 0), stop=(kt == DT - 1))
                for j in range(4):
                    st = 4 * c + j
                    nc.vector.tensor_scalar_mul(
                        out=v_sb[:, st, :], in0=pts[j][:],
                        scalar1=s_tok[:, st:st + 1])
                # Q/K projections + RoPE
                for ot in range(8):
                    wt = wkp.tile([P, DT, P], CDT, tag="wk")
                    nc.sync.dma_start(out=wt[:], in_=w_qk[ot])
                    pt = ps_qk.tile([P, CW], f32, tag="pqk")
                    for kt in range(DT):
                        nc.tensor.matmul(pt[:], wt[:, kt], zt[:, kt, :],
                                         start=(kt == 0), stop=(kt == DT - 1))
                    swp = ropep.tile([P, CW], CDT, tag="swp")
                    nc.vector.tensor_copy(swp[0:64, :], pt[64:128, :])
                    nc.vector.tensor_copy(swp[64:128, :], pt[0:64, :])
                    t1 = ropep.tile([P, CW], CDT, tag="t1")
                    nc.vector.tensor_mul(t1[:], pt[:], cs_sb[:, ch(c)])
                    nc.vector.tensor_mul(swp[:], swp[:], sn_sb[:, ch(c)])
                    hh = ot % 4
                    if ot < 4:
                        nc.vector.tensor_add(q_t[:, hh, :], t1[:], swp[:])
                    else:
                        nc.vector.tensor_add(kts[hh][:, ch(c)], t1[:], swp[:])

        def attn_chunk(c, q_t):
            """attention for query chunk c (heads in pairs); wo; RS(y_c)."""
            at_t = atp.tile([P, NHC, CW], CDT, tag="att")
            nkt = 4 * c + 4
            with nc.named_scope(f"attn_c{c}"):
                for h0 in (0, 2):
                    avs, esums = [], []
                    for hh in (h0, h0 + 1):
                        avs.append(ps_av.tile([P, CW], f32, tag="pav",
                                              name=f"av{hh}"))
                        esums.append(esump.tile([P, CW], f32, tag="esum",
                                                name=f"es{hh}"))
                    for ktile in range(nkt):
                        diag = ktile >= 4 * c
                        col0 = P * (ktile - 4 * c) if diag else 0
                        for j, hh in enumerate((h0, h0 + 1)):
                            scp = ps_sc.tile([P, CW], f32, tag="scp")
                            nc.tensor.matmul(
                                scp[:, col0:],
                                kts[hh][:, P * ktile:P * (ktile + 1)],
                                q_t[:, hh, col0:],
                                start=True, stop=True)
                            if diag:
                                nc.vector.tensor_add(
                                    scp[:, col0:col0 + P],
                                    scp[:, col0:col0 + P], dmask_sb[:])
                            et = expp.tile([P, CW], CDT, tag="et")
                            if col0 > 0:
                                nc.vector.memset(et[:, 0:col0], 0.0)
                            nc.scalar.activation(out=et[:, col0:],
                                                 in_=scp[:, col0:],
                                                 func=EXP, scale=ISQ)
                            if ktile == 0:
                                nc.vector.tensor_copy(out=esums[j][:],
                                                      in_=et[:])
                            else:
                                nc.vector.tensor_add(esums[j][:],
                                                     esums[j][:], et[:])
                            nc.tensor.matmul(
                                avs[j][:],
                                v_sb[:, ktile, HD * hh:HD * (hh + 1)],
                                et[:], start=(ktile == 0),
                                stop=(ktile == nkt - 1))
                    for j, hh in enumerate((h0, h0 + 1)):
                        dps = ps_sc.tile([P, CW], f32, tag="scp")
                        nc.tensor.matmul(dps[0:1, :], ones_f[:],
                                         esums[j][:], start=True, stop=True)
                        rec = recp.tile([1, CW], f32, tag="rc")
                        nc.vector.reciprocal_approx_fast(out=rec[:],
                                                         in_=dps[0:1, :])
                        rrep = repp.tile([P, CW], f32, tag="rrep")
                        nc.gpsimd.partition_broadcast(rrep[:], rec[:])
                        nc.vector.tensor_mul(at_t[:, hh, :], avs[j][:],
                                             rrep[:])
                # ---- wo for this chunk, then RS it ----
                for ot in range(32):
                    wo_t = wop.tile([P, 4, P], CDT, tag="wo")
                    nc.scalar.dma_start(out=wo_t[:], in_=w_o[:, ot])
                    pt = ps_av.tile([P, CW], f32, tag="pav")
                    for dt_i in range(4):
                        nc.tensor.matmul(pt[:], wo_t[:, dt_i],
                                         at_t[:, dt_i, :],
                                         start=(dt_i == 0), stop=(dt_i == 3))
                    yt = ytp.tile([P, CW], CDT, tag="yt")
                    if ot % 2 == 0:
                        nc.vector.tensor_copy(out=yt[:], in_=pt[:])
                    else:
                        nc.scalar.copy(out=yt[:], in_=pt[:])
                    nc.sync.dma_start(out=yT_cc[c][P * ot:P * (ot + 1), :],
                                      in_=yt[:])
                nc.gpsimd.collective_compute(
                    "ReduceScatter", ADD, ins=[yT_cc[c][:]],
                    outs=[y_rs[c][:]], replica_groups=RG)

        def stage4_chunk(c):
            """h = x + y (bf16) and AG(h_c).  No Tensor-queue ops."""
            with nc.named_scope(f"resid_c{c}"):
                for i in range(4):
                    xb = resid.tile([P, CW], CDT, tag="xb")
                    nc.scalar.dma_start(out=xb[:],
                                        in_=xT_bf[P * i:P * (i + 1), ch(c)])
                    ys = resid.tile([P, CW], CDT, tag="ys")
                    nc.gpsimd.dma_start(out=ys[:],
                                        in_=y_rs[c][P * i:P * (i + 1), :])
                    nc.vector.tensor_add(hT_bf[i][:, ch(c)], xb[:], ys[:])
                    nc.scalar.dma_start(out=hn_cc[c][P * i:P * (i + 1), :],
                                        in_=hT_bf[i][:, ch(c)])
                nc.gpsimd.collective_compute(
                    "AllGather", BYP, ins=[hn_cc[c][:]], outs=[hnT_ag[c][:]],
                    replica_groups=RG)

        def ffn_chunk(c):
            """SwiGLU FFN for chunk c (stats from gathered h); RS'd."""
            with nc.named_scope(f"ffn_c{c}"):
                hn_sb = xinp.tile([P, DT, CW], CDT, tag="xin", name="hn")
                hv = hnT_ag[c][:].rearrange("(kt p) s -> p kt s", p=P)
                for q4 in range(4):
                    nc.sync.dma_start(out=hn_sb[:, 8 * q4:8 * (q4 + 1), :],
                                      in_=hv[:, 8 * q4:8 * (q4 + 1), :])
                # local stats for the ffn norm
                spt = ps_sc.tile([P, CW], f32, tag="scp")
                for kt in range(DT):
                    sq = sqp.tile([P, CW], CDT, tag="sq")
                    nc.vector.tensor_mul(sq[:], hn_sb[:, kt, :],
                                         hn_sb[:, kt, :])
                    nc.tensor.matmul(spt[0:1, :], ones[:], sq[:],
                                     start=(kt == 0), stop=(kt == DT - 1))
                s2c = repp.tile([P, CW], f32, tag="rrep", name="s2c")
                rms_scale(spt[0:1, :], s2c[:])
                nc.vector.tensor_copy(out=s2rep[:, ch(c)], in_=s2c[:])
                g_sb = gp.tile([P, FT, CW], CDT, tag="g")
                for ft in range(FT):
                    w1t = wkp.tile([P, DT, P], CDT, tag="wk")
                    nc.sync.dma_start(out=w1t[:], in_=w_1[ft])
                    w3t = wkp.tile([P, DT, P], CDT, tag="wk")
                    nc.scalar.dma_start(out=w3t[:], in_=w_3[ft])
                    p1 = ps_sc.tile([P, CW], f32, tag="scp")
                    for kt in range(DT):
                        nc.tensor.matmul(p1[:], w1t[:, kt], hn_sb[:, kt, :],
                                         start=(kt == 0), stop=(kt == DT - 1))
                    p3 = ps_sc.tile([P, CW], f32, tag="scp")
                    for kt in range(DT):
                        nc.tensor.matmul(p3[:], w3t[:, kt], hn_sb[:, kt, :],
                                         start=(kt == 0), stop=(kt == DT - 1))
                    t1s = fsp.tile([P, CW], CDT, tag="t1s")
                    nc.vector.tensor_mul(t1s[:], p1[:], s2rep[:, ch(c)])
                    tsi = fsp.tile([P, CW], CDT, tag="tsi")
                    nc.scalar.activation(out=tsi[:], in_=t1s[:], func=SILU)
                    t3s = fsp.tile([P, CW], CDT, tag="t3s")
                    nc.vector.tensor_mul(t3s[:], p3[:], s2rep[:, ch(c)])
                    nc.vector.tensor_mul(g_sb[:, ft, :], t3s[:], tsi[:])
                # down-proj; chunk 3 split into two half-token groups
                parts = [(c, 0, CW)] if c < 3 else [(3, 0, HW), (4, HW, HW)]
                for k_o, off, w in parts:
                    for ot in range(32):
                        w2t = w2p.tile([P, FT, P], CDT, tag="w2")
                        nc.sync.dma_start(out=w2t[:], in_=w_2[ot])
                        pt = ps_av.tile([P, CW], f32, tag="pav")
                        for ft in range(FT):
                            nc.tensor.matmul(pt[:, 0:w], w2t[:, ft],
                                             g_sb[:, ft, off:off + w],
                                             start=(ft == 0),
                                             stop=(ft == FT - 1))
                        og = ogp.tile([P, CW], CDT, tag="og")
                        if ot % 2 == 0:
                            nc.vector.tensor_copy(out=og[:, 0:w],
                                                  in_=pt[:, 0:w])
                        else:
                            nc.scalar.copy(out=og[:, 0:w], in_=pt[:, 0:w])
                        nc.sync.dma_start(
                            out=oT_cc[k_o][P * ot:P * (ot + 1), :],
                            in_=og[:, 0:w])
                    nc.gpsimd.collective_compute(
                        "ReduceScatter", ADD, ins=[oT_cc[k_o][:]],
                        outs=[o_rs[k_o][:]], replica_groups=RG)

        def stage6_chunk(k_o):
            """out = h + o for o-chunk k_o (f32)."""
            _, off, w = OCH[k_o]
            with nc.named_scope(f"out_c{k_o}"):
                for i in range(4):
                    ob = resid.tile([P, CW], CDT, tag="ob")
                    nc.gpsimd.dma_start(out=ob[:, 0:w],
                                        in_=o_rs[k_o][P * i:P * (i + 1), :])
                    o6 = resid.tile([P, CW], f32, tag="o6", bufs=1)
                    nc.vector.tensor_add(o6[:, 0:w],
                                         hT_bf[i][:, off:off + w],
                                         ob[:, 0:w])
                    nc.sync.dma_start(
                        out=outT_s[P * i:P * (i + 1), off:off + w],
                        in_=o6[:, 0:w])

        # ================= emission schedule =================
        # stage4(c) is emitted one step late (after qkv(c+1); after ffn(0)
        # for the last chunk) so nothing behind it waits on RS(y_c).
        q0 = qp.tile([P, NHC, CW], CDT, tag="qt")
        qkv_chunk(0, q0)
        attn_chunk(0, q0)
        for c in range(1, NCH):
            q_t = qp.tile([P, NHC, CW], CDT, tag="qt")
            qkv_chunk(c, q_t)
            stage4_chunk(c - 1)
            attn_chunk(c, q_t)

        ffn_chunk(0)
        stage4_chunk(3)
        ffn_chunk(1)
        stage6_chunk(0)
        ffn_chunk(2)
        stage6_chunk(1)
        ffn_chunk(3)
        stage6_chunk(2)
        stage6_chunk(3)
        stage6_chunk(4)

        for cm in (resid_cm, og_cm, fs_cm, w2_cm, g_cm, yt_cm, rep_cm,
                   rec_cm, esum_cm, exp_cm, at_cm, sq_cm, q_cm,
                   rope_cm, wo_cm, wv_cm, wk_cm, xin_cm):
            cm.__exit__(None, None, None)
        es.close()

    nc.compile()
    return nc


def _prep_inputs(x, freqs_cos, freqs_sin, mask, attn_norm_w, wq, wk, wv, wo,
                 ffn_norm_w, w1, w2, w3):
    """Host-side sharding + weight layout. Returns in_maps for 8 cores."""
    f32 = np.float32
    x2 = np.asarray(x, f32)[0]                     # [S, D]
    xT = np.ascontiguousarray(x2.T)                # [D, S]
    anw = np.asarray(attn_norm_w, f32)
    fnw = np.asarray(ffn_norm_w, f32)
    wq = np.asarray(wq, f32) * anw[None, :]
    wk = np.asarray(wk, f32) * anw[None, :]
    wv_e = np.asarray(wv, f32)
    wo = np.asarray(wo, f32)
    w1 = np.asarray(w1, f32) * fnw[None, :]
    w3 = np.asarray(w3, f32) * fnw[None, :]
    w2 = np.asarray(w2, f32)

    perm = np.concatenate([np.arange(0, HD, 2), np.arange(1, HD, 2)])

    cosT = np.ascontiguousarray(np.asarray(freqs_cos, f32).T)   # [64, S]
    sinT = np.ascontiguousarray(np.asarray(freqs_sin, f32).T)
    cos2 = np.concatenate([cosT, cosT], axis=0).astype(NP_CDT)  # [128, S]
    sinsg2 = np.concatenate([-sinT, sinT], axis=0).astype(NP_CDT)
    m = np.asarray(mask, f32)[0, 0]
    dmask = (np.ascontiguousarray(m[:P, :P].T) * f32(math.sqrt(HD))).astype(f32)

    def lhsT_tiles(wt, n_out_tiles, n_k_tiles):
        # wt: [K, Mout] -> [ot, p, kt, j] with [ot,p,kt,j] = wt[128*kt+p, 128*ot+j]
        a = wt.reshape(n_k_tiles, P, n_out_tiles, P)
        return np.ascontiguousarray(a.transpose(2, 1, 0, 3)).astype(NP_CDT)

    in_maps = []
    for r in range(CORES):
        ds = slice(DQ * r, DQ * (r + 1))
        wqT = wq[ds].T.copy()                      # [D, DQ]
        wkT = wk[ds].T.copy()
        for h in range(NHC):
            blk = slice(HD * h, HD * (h + 1))
            wqT[:, blk] = wqT[:, blk][:, perm]
            wkT[:, blk] = wkT[:, blk][:, perm]
        wqk = np.concatenate([lhsT_tiles(wqT, NHC, DT),
                              lhsT_tiles(wkT, NHC, DT)], axis=0)  # [8,P,DT,P]
        wvT = wv_e[ds].T.copy()                    # [D, DQ]
        w_v_l = np.ascontiguousarray(wvT.reshape(DT, P, DQ)).astype(NP_CDT)
        woT = wo[:, ds].T.copy()                   # [DQ, D]
        wo_l = lhsT_tiles(woT, 32, 4)              # [32, P, 4, P]
        wo_l = np.ascontiguousarray(wo_l.transpose(1, 0, 2, 3))  # [P,32,4,P]
        fs = slice(FC * r, FC * (r + 1))
        w1s = np.zeros((FP, D), f32)
        w3s = np.zeros((FP, D), f32)
        w1s[:FC] = w1[fs]
        w3s[:FC] = w3[fs]
        w1_l = lhsT_tiles(np.ascontiguousarray(w1s.T), FT, DT)  # [FT, P, DT, P]
        w3_l = lhsT_tiles(np.ascontiguousarray(w3s.T), FT, DT)
        w2s = np.zeros((FP, D), f32)
        w2s[:FC] = w2[:, fs].T                     # [FP, D] (rows = f)
        w2_l = lhsT_tiles(w2s, 32, FT)             # [32, P, FT, P]

        in_maps.append({
            "xT_bf": np.ascontiguousarray(xT[ds]).astype(NP_CDT),
            "w_qk": wqk,
            "w_v": w_v_l,
            "w_o": wo_l,
            "w_1": w1_l,
            "w_3": w3_l,
            "w_2": w2_l,
            "cos2": cos2,
            "sinsg2": sinsg2,
            "dmask": dmask,
        })
    return in_maps


def kernel(x, freqs_cos, freqs_sin, mask, attn_norm_w, wq, wk, wv, wo,
           ffn_norm_w, w1, w2, w3, _trace=False):
    global _COMPILED
    if _COMPILED is None:
        _COMPILED = _build()
    nc = _COMPILED
    in_maps = _prep_inputs(x, freqs_cos, freqs_sin, mask, attn_norm_w,
                           wq, wk, wv, wo, ffn_norm_w, w1, w2, w3)
    res = run_bass_kernel_spmd(nc, in_maps, list(range(CORES)), trace=_trace)
    kernel.last_result = res
    outT = np.concatenate([res.results[r]["outT_s"] for r in range(CORES)],
                          axis=0)                  # [D, S]
    return np.ascontiguousarray(outT.T)[None].astype(np.float32)


# revision 37
# speedup vs baseline: 1.1597x; 1.1597x over previous
"""Llama-style transformer block on 8 TRN2 NeuronCores.

Megatron tensor-parallel with feature-major (transposed) activations:
  - Residual stream kept TRANSPOSED (x^T: [D, S]) so every matmul contracts
    over the partition dim with zero on-chip transposes.
  - Per core: 4 attention heads (512 of 4096 q/k/v dims) and 1376 (padded
    to 1408) of the 11008 FFN hidden dims.
  - RMSNorm: per-core partial sum-of-squares over the 512-feature shard,
    AllReduce [1,2048], scale own shard, AllGather normalized activations
    (feature-stacked = the exact layout the matmuls consume).
  - Attention: transposed scores ([s_k, s_q]) feed the AV matmul directly;
    softmax skips max-subtraction (exact; scores bounded).  Denominators
    via ones-matmul; normalization deferred to attention output.
  - RoPE via host-side even/odd permutation of head dims + two aligned
    half-tile swaps against [cos;cos] / [-sin;sin] tables.
  - wo / w2 partial sums -> token-chunked ReduceScatter, pipelined under
    the next chunk's compute.  AllGathers similarly chunked.

v2: everything pipelined in 512-token chunks to keep TensorE continuously
busy (HAM clock stays warm) and hide collectives under compute.
"""

import math

import ml_dtypes
import numpy as np

import concourse.bass as bass
import concourse.mybir as mybir
import concourse.tile as tile
from concourse import bacc
from concourse.bass_utils import run_bass_kernel_spmd

S = 2048
D = 4096
HD = 128
NH = 32
F = 11008
CORES = 8
NHC = NH // CORES          # heads per core = 4
DQ = NHC * HD              # q/k/v dims per core = 512
FC = F // CORES            # ffn dims per core = 1376
FT = 11                    # padded f-tiles per core
FP = FT * 128
EPS = 1e-5
P = 128
NCH = 4                    # 512-token chunks
CW = S // NCH              # chunk width = 512
DT = D // P                # d tiles = 32
ST = S // P                # s tiles = 16

CDT = mybir.dt.bfloat16
NP_CDT = ml_dtypes.bfloat16

_COMPILED = None


def _build():
    nc = bacc.Bacc("TRN2", target_bir_lowering=False, debug=False,
                   num_devices=CORES)
    f32 = mybir.dt.float32

    # ---- kernel I/O ----
    xT_s = nc.declare_dram_parameter("xT_s", [DQ, S], f32, isOutput=False)
    xT_bf = nc.declare_dram_parameter("xT_bf", [DQ, S], CDT, isOutput=False)
    w_qk = nc.declare_dram_parameter("w_qk", [8, P, DT, P], CDT, isOutput=False)
    w_v = nc.declare_dram_parameter("w_v", [DT, P, DQ], CDT, isOutput=False)
    w_o = nc.declare_dram_parameter("w_o", [P, 32, 4, P], CDT, isOutput=False)
    w_1 = nc.declare_dram_parameter("w_1", [FT, P, DT, P], CDT, isOutput=False)
    w_3 = nc.declare_dram_parameter("w_3", [FT, P, DT, P], CDT, isOutput=False)
    w_2 = nc.declare_dram_parameter("w_2", [32, P, FT, P], CDT, isOutput=False)
    cos2 = nc.declare_dram_parameter("cos2", [P, S], CDT, isOutput=False)
    sinsg2 = nc.declare_dram_parameter("sinsg2", [P, S], CDT, isOutput=False)
    dmask = nc.declare_dram_parameter("dmask", [P, P], f32, isOutput=False)
    outT_s = nc.declare_dram_parameter("outT_s", [DQ, S], f32, isOutput=True)

    # ---- internal DRAM ----
    ssq1_in = nc.dram_tensor("ssq1_in", [1, S], f32)
    s1_scr = nc.dram_tensor("s1_scr", [1, S], f32)
    ssq1_out = nc.dram_tensor("ssq1_out", [1, S], f32, addr_space="Shared")
    zs_cc = [nc.dram_tensor(f"zs_cc{c}", [DQ, CW], CDT) for c in range(NCH)]
    zT_ag = [nc.dram_tensor(f"zT_ag{c}", [D, CW], CDT, addr_space="Shared")
             for c in range(NCH)]
    qt_dram = nc.dram_tensor("qt_dram", [DQ, S], CDT)
    kt_dram = nc.dram_tensor("kt_dram", [DQ, S], CDT)
    yT_cc = [nc.dram_tensor(f"yT_cc{c}", [D, CW], CDT) for c in range(NCH)]
    y_rs = [nc.dram_tensor(f"y_rs{c}", [DQ, CW], CDT) for c in range(NCH)]
    ssq2_in = nc.dram_tensor("ssq2_in", [1, S], f32)
    ssq2_out = nc.dram_tensor("ssq2_out", [1, S], f32, addr_space="Shared")
    hn_cc = [nc.dram_tensor(f"hn_cc{c}", [DQ, CW], CDT) for c in range(NCH)]
    hnT_ag = [nc.dram_tensor(f"hnT_ag{c}", [D, CW], CDT, addr_space="Shared")
              for c in range(NCH)]
    oT_cc = [nc.dram_tensor(f"oT_cc{c}", [D, CW], CDT) for c in range(NCH)]
    o_rs = [nc.dram_tensor(f"o_rs{c}", [DQ, CW], CDT) for c in range(NCH)]

    RG = [list(range(CORES))]
    ADD = mybir.AluOpType.add
    BYP = mybir.AluOpType.bypass
    EXP = mybir.ActivationFunctionType.Exp
    SQRT = mybir.ActivationFunctionType.Sqrt
    SILU = mybir.ActivationFunctionType.Silu
    ISQ = 1.0 / math.sqrt(HD)

    def ch(c):
        return slice(CW * c, CW * (c + 1))

    with tile.TileContext(nc) as tc:
        with (
            tc.tile_pool(name="persist", bufs=1) as persist,
            tc.tile_pool(name="ps_small", bufs=1, space="PSUM") as ps_small,
        ):
            ones = persist.tile([P, 1], CDT)
            nc.vector.memset(ones[:], 1.0)
            eps_sb = persist.tile([P, 1], f32)
            nc.vector.memset(eps_sb[:], EPS)
            dmask_sb = persist.tile([P, P], f32)
            nc.sync.dma_start(out=dmask_sb[:], in_=dmask[:])
            hT = [persist.tile([P, S], f32, tag=f"hT{i}", name=f"hT{i}")
                  for i in range(4)]
            cs_sb = persist.tile([P, S], CDT)
            sn_sb = persist.tile([P, S], CDT)
            s_tok = persist.tile([P, ST], f32)
            s2rep = persist.tile([P, S], f32)

            # raw x^T shard -> collective bounce buffers, first thing: the
            # AllGathers of raw x can then fire as soon as the stats
            # AllReduce has been triggered (norm scales are applied
            # post-matmul, so the gather does not wait for the norm).
            for c in range(NCH):
                nc.sync.dma_start(out=zs_cc[c][:], in_=xT_bf[:, ch(c)])

            # stage-1 weight pool opened early so chunk-0 Q/K weights
            # prefetch during the norm + AllReduce window
            st1w_cm = tc.tile_pool(name="st1w", bufs=3)
            st1w = st1w_cm.__enter__()
            pre_wt = {}
            for ot in range(3):
                wt = st1w.tile([P, DT, P], CDT, tag="wqk", name=f"prew{ot}")
                nc.sync.dma_start(out=wt[:], in_=w_qk[ot])
                pre_wt[ot] = wt

            # ============ stage 0: attn RMSNorm + AllGather(z) ============
            with tc.tile_pool(name="st0", bufs=1) as st0:
                xt = []
                for i in range(4):
                    t = st0.tile([P, S], CDT, tag=f"xt{i}")
                    nc.sync.dma_start(out=t[:], in_=xT_bf[P * i:P * (i + 1), :])
                    xt.append(t)
                sq = []
                for i in range(4):
                    t = st0.tile([P, S], CDT, tag=f"sq{i}")
                    nc.vector.tensor_mul(t[:], xt[i][:], xt[i][:])
                    sq.append(t)
                ssq_sb = st0.tile([1, S], f32)
                for c in range(NCH):
                    pt = ps_small.tile([1, CW], f32, tag="one512")
                    for i in range(4):
                        nc.tensor.matmul(pt[:], ones[:], sq[i][:, ch(c)],
                                         start=(i == 0), stop=(i == 3))
                    nc.any.tensor_copy(out=ssq_sb[:, ch(c)], in_=pt[:])
                nc.gpsimd.dma_start(out=ssq1_in[:], in_=ssq_sb[:])
                nc.gpsimd.collective_compute(
                    "AllReduce", ADD, ins=[ssq1_in[:]], outs=[ssq1_out[:]],
                    replica_groups=RG)
                for c in range(NCH):
                    nc.gpsimd.collective_compute(
                        "AllGather", BYP, ins=[zs_cc[c][:]], outs=[zT_ag[c][:]],
                        replica_groups=RG)
                # s1 = rsqrt(mean+eps); fold into RoPE tables (Q,K) and a
                # token-major per-partition scale for V
                sg_sb = st0.tile([1, S], f32)
                nc.gpsimd.dma_start(out=sg_sb[:], in_=ssq1_out[:])
                sr_f = st0.tile([1, S], f32)
                nc.scalar.activation(out=sr_f[:], in_=sg_sb[:], func=SQRT,
                                     bias=eps_sb[0:1], scale=1.0 / D)
                nc.vector.reciprocal(out=sr_f[:], in_=sr_f[:])
                s1rep = st0.tile([P, S], f32)
                nc.gpsimd.partition_broadcast(s1rep[:], sr_f[:])
                cos_raw = st0.tile([P, S], CDT, tag="cosr")
                sin_raw = st0.tile([P, S], CDT, tag="sinr")
                nc.sync.dma_start(out=cos_raw[:], in_=cos2[:])
                nc.sync.dma_start(out=sin_raw[:], in_=sinsg2[:])
                nc.vector.tensor_mul(cs_sb[:], cos_raw[:], s1rep[:])
                nc.vector.tensor_mul(sn_sb[:], sin_raw[:], s1rep[:])
                nc.gpsimd.dma_start(out=s1_scr[:], in_=sr_f[:])
                nc.sync.dma_start(
                    out=s_tok[:],
                    in_=s1_scr[:].rearrange("o (st p) -> p (o st)", p=P))

            with tc.tile_pool(name="attn_persist", bufs=1) as apst:
                attnT = apst.tile([P, NHC, S], CDT)
                v_sb = apst.tile([P, ST, DQ], CDT)

                # ===== stage 1: Q/K/V projections (+RoPE), per 512-chunk ====
                with (
                    tc.tile_pool(name="st1", bufs=1) as st1,
                    tc.tile_pool(name="st1z", bufs=2) as st1z,
                    tc.tile_pool(name="rope", bufs=3) as rope,
                    tc.tile_pool(name="ps_qkv", bufs=2, space="PSUM") as ps_qkv,
                    tc.tile_pool(name="ps_v", bufs=1, space="PSUM") as ps_v,
                ):
                    for c in range(NCH):
                      with nc.named_scope(f"qkv_c{c}"):
                        zt = st1z.tile([P, DT, CW], CDT, tag="zt")
                        zv = zT_ag[c][:].rearrange("(kt p) s -> p kt s", p=P)
                        for q4 in range(4):
                            nc.sync.dma_start(out=zt[:, 8 * q4:8 * (q4 + 1), :],
                                              in_=zv[:, 8 * q4:8 * (q4 + 1), :])
                        # --- Q and K ---
                        for ot in range(8):
                            if c == 0 and ot in pre_wt:
                                wt = pre_wt[ot]
                            else:
                                wt = st1w.tile([P, DT, P], CDT, tag="wqk")
                                nc.sync.dma_start(out=wt[:], in_=w_qk[ot])
                            pt = ps_qkv.tile([P, CW], f32, tag="pqk")
                            for kt in range(DT):
                                nc.tensor.matmul(pt[:], wt[:, kt], zt[:, kt, :],
                                                 start=(kt == 0), stop=(kt == DT - 1))
                            # RoPE: out = pt*[c;c] + swap(pt)*[-s;s]
                            swp = rope.tile([P, CW], f32, tag="swp")
                            nc.vector.tensor_copy(swp[0:64, :], pt[64:128, :])
                            nc.vector.tensor_copy(swp[64:128, :], pt[0:64, :])
                            t1 = rope.tile([P, CW], f32, tag="t1")
                            t2 = rope.tile([P, CW], f32, tag="t2")
                            nc.vector.tensor_mul(t1[:], pt[:], cs_sb[:, ch(c)])
                            nc.vector.tensor_mul(t2[:], swp[:], sn_sb[:, ch(c)])
                            qk = rope.tile([P, CW], CDT, tag="qk")
                            nc.vector.tensor_add(qk[:], t1[:], t2[:])
                            dst = qt_dram if ot < 4 else kt_dram
                            hh = ot % 4
                            nc.sync.dma_start(out=dst[P * hh:P * (hh + 1), ch(c)],
                                              in_=qk[:])
                        # --- V: 4 token-tiles of this chunk ---
                        pts = [ps_v.tile([P, DQ], f32, tag=f"pv{i}", name=f"pv{i}")
                               for i in range(4)]
                        for kt in range(DT):
                            wv = st1w.tile([P, DQ], CDT, tag="wv")
                            nc.sync.dma_start(out=wv[:], in_=w_v[kt])
                            for i in range(4):
                                tok = slice(P * i, P * (i + 1))
                                nc.tensor.matmul(
                                    pts[i][:], zt[:, kt, tok], wv[:],
                                    start=(kt == 0), stop=(kt == DT - 1))
                        for i in range(4):
                            st = 4 * c + i
                            nc.vector.tensor_scalar_mul(
                                out=v_sb[:, st, :], in0=pts[i][:],
                                scalar1=s_tok[:, st:st + 1])

                # ====== stage 2+3: attention + wo + chunked RS(y) ======
                with (
                    tc.tile_pool(name="st2", bufs=3) as st2,
                    tc.tile_pool(name="st2qk", bufs=1) as st2qk,
                    tc.tile_pool(name="st2y", bufs=6) as st2y,
                    tc.tile_pool(name="exps", bufs=6) as exps,
                    tc.tile_pool(name="ps_sc", bufs=3, space="PSUM") as ps_sc,
                    tc.tile_pool(name="ps_av", bufs=2, space="PSUM") as ps_av,
                    tc.tile_pool(name="ps_wo", bufs=2, space="PSUM") as ps_wo,
                ):
                    wo_sb = st2qk.tile([P, 32, 4, P], CDT)
                    nc.sync.dma_start(out=wo_sb[:], in_=w_o[:])
                    qts, kts = [], []
                    for hh in range(NHC):
                        qt = st2qk.tile([P, S], CDT, tag=f"qt{hh}", name=f"qt{hh}")
                        kt_t = st2qk.tile([P, S], CDT, tag=f"kt{hh}", name=f"kt{hh}")
                        for cq in range(NCH):
                            nc.sync.dma_start(
                                out=qt[:, ch(cq)],
                                in_=qt_dram[P * hh:P * (hh + 1), ch(cq)])
                            nc.sync.dma_start(
                                out=kt_t[:, ch(cq)],
                                in_=kt_dram[P * hh:P * (hh + 1), ch(cq)])
                        qts.append(qt)
                        kts.append(kt_t)
                    for qc in range(NCH):
                      with nc.named_scope(f"attn_c{qc}"):
                        nkt = 4 * qc + 4
                        for hh in range(NHC):
                            qt, kt_t = qts[hh], kts[hh]
                            avp = ps_av.tile([P, CW], f32, tag="avp")
                            smp = ps_small.tile([1, CW], f32, tag="one512")
                            for ktile in range(nkt):
                                diag = ktile >= 4 * qc
                                col0 = P * (ktile - 4 * qc) if diag else 0
                                scp = ps_sc.tile([P, CW], f32, tag="scp")
                                nc.tensor.matmul(
                                    scp[:, col0:],
                                    kt_t[:, P * ktile:P * (ktile + 1)],
                                    qt[:, CW * qc + col0:CW * (qc + 1)],
                                    start=True, stop=True)
                                if diag:
                                    nc.vector.tensor_add(
                                        scp[:, col0:col0 + P],
                                        scp[:, col0:col0 + P], dmask_sb[:])
                                et = exps.tile([P, CW], CDT, tag="et")
                                if col0 > 0:
                                    nc.vector.memset(et[:, 0:col0], 0.0)
                                nc.scalar.activation(out=et[:, col0:],
                                                     in_=scp[:, col0:],
                                                     func=EXP, scale=ISQ)
                                nc.tensor.matmul(
                                    avp[:], v_sb[:, ktile, P * hh:P * (hh + 1)],
                                    et[:], start=(ktile == 0),
                                    stop=(ktile == nkt - 1))
                                nc.tensor.matmul(smp[:], ones[:], et[:],
                                                 start=(ktile == 0),
                                                 stop=(ktile == nkt - 1))
                            rec = st2.tile([1, CW], f32, tag="rec")
                            nc.vector.reciprocal(out=rec[:], in_=smp[:])
                            rrep = st2.tile([P, CW], f32, tag="rrep")
                            nc.gpsimd.partition_broadcast(rrep[:], rec[:])
                            nc.vector.tensor_mul(attnT[:, hh, ch(qc)], avp[:],
                                                 rrep[:])
                        # ---- wo for this chunk, then RS it ----
                        for ot in range(32):
                            pt = ps_wo.tile([P, CW], f32, tag="pwo")
                            for dt_i in range(4):
                                nc.tensor.matmul(pt[:], wo_sb[:, ot, dt_i],
                                                 attnT[:, dt_i, ch(qc)],
                                                 start=(dt_i == 0), stop=(dt_i == 3))
                            yt = st2y.tile([P, CW], CDT, tag="yt")
                            if ot % 2 == 0:
                                nc.vector.tensor_copy(out=yt[:], in_=pt[:])
                            else:
                                nc.scalar.copy(out=yt[:], in_=pt[:])
                            nc.sync.dma_start(out=yT_cc[qc][P * ot:P * (ot + 1), :],
                                              in_=yt[:])
                        nc.gpsimd.collective_compute(
                            "ReduceScatter", ADD, ins=[yT_cc[qc][:]],
                            outs=[y_rs[qc][:]], replica_groups=RG)

            st1w_cm.__exit__(None, None, None)

            # ====== stage 4: residual + stats + chunked AG(raw h) ======
            with tc.tile_pool(name="st4", bufs=2) as st4:
                ssq_sb2 = persist.tile([1, S], f32)
                for c in range(NCH):
                    sq2 = []
                    for i in range(4):
                        xt_i = st4.tile([P, CW], f32, tag="x4")
                        nc.sync.dma_start(out=xt_i[:],
                                          in_=xT_s[P * i:P * (i + 1), ch(c)])
                        ys = st4.tile([P, CW], CDT, tag="ys")
                        nc.gpsimd.dma_start(out=ys[:],
                                            in_=y_rs[c][P * i:P * (i + 1), :])
                        nc.vector.tensor_add(hT[i][:, ch(c)], xt_i[:], ys[:])
                        t = st4.tile([P, CW], CDT, tag="sq2")
                        nc.vector.tensor_mul(t[:], hT[i][:, ch(c)], hT[i][:, ch(c)])
                        sq2.append(t)
                        hb = st4.tile([P, CW], CDT, tag="hb")
                        nc.vector.tensor_copy(out=hb[:], in_=hT[i][:, ch(c)])
                        nc.gpsimd.dma_start(out=hn_cc[c][P * i:P * (i + 1), :],
                                            in_=hb[:])
                    pt = ps_small.tile([1, CW], f32, tag="one512")
                    for i in range(4):
                        nc.tensor.matmul(pt[:], ones[:], sq2[i][:],
                                         start=(i == 0), stop=(i == 3))
                    nc.any.tensor_copy(out=ssq_sb2[:, ch(c)], in_=pt[:])
                    if c == NCH - 1:
                        # AllReduce of the stats goes on the stream before the
                        # last h AllGather so s2 is ready when FFN needs it
                        nc.gpsimd.dma_start(out=ssq2_in[:], in_=ssq_sb2[:])
                        nc.gpsimd.collective_compute(
                            "AllReduce", ADD, ins=[ssq2_in[:]],
                            outs=[ssq2_out[:]], replica_groups=RG)
                    nc.gpsimd.collective_compute(
                        "AllGather", BYP, ins=[hn_cc[c][:]], outs=[hnT_ag[c][:]],
                        replica_groups=RG)
                sg2_sb = st4.tile([1, S], f32)
                nc.gpsimd.dma_start(out=sg2_sb[:], in_=ssq2_out[:])
                sr2_f = st4.tile([1, S], f32)
                nc.scalar.activation(out=sr2_f[:], in_=sg2_sb[:], func=SQRT,
                                     bias=eps_sb[0:1], scale=1.0 / D)
                nc.vector.reciprocal(out=sr2_f[:], in_=sr2_f[:])
                nc.gpsimd.partition_broadcast(s2rep[:], sr2_f[:])

            # ============ stage 5: FFN + chunked RS(o) ============
            with (
                tc.tile_pool(name="st5w", bufs=2) as st5w,
                tc.tile_pool(name="st5w2", bufs=3) as st5w2,
                tc.tile_pool(name="st5h", bufs=2) as st5h,
                tc.tile_pool(name="st5g", bufs=2) as st5g,
                tc.tile_pool(name="st5t", bufs=4) as st5t,
                tc.tile_pool(name="ps_f1", bufs=2, space="PSUM") as ps_f1,
                tc.tile_pool(name="ps_f3", bufs=2, space="PSUM") as ps_f3,
                tc.tile_pool(name="ps_w2", bufs=2, space="PSUM") as ps_w2,
            ):
                for c in range(NCH):
                    with nc.named_scope(f"ffn_c{c}"):
                        hn_sb = st5h.tile([P, DT, CW], CDT, tag="hn")
                        hv = hnT_ag[c][:].rearrange("(kt p) s -> p kt s", p=P)
                        for q4 in range(4):
                            nc.sync.dma_start(out=hn_sb[:, 8 * q4:8 * (q4 + 1), :],
                                              in_=hv[:, 8 * q4:8 * (q4 + 1), :])
                        g_sb = st5g.tile([P, FT, CW], CDT, tag="g")
                        for ft in range(FT):
                            w1t = st5w.tile([P, DT, P], CDT, tag="w1")
                            w3t = st5w.tile([P, DT, P], CDT, tag="w3")
                            nc.sync.dma_start(out=w1t[:], in_=w_1[ft])
                            nc.sync.dma_start(out=w3t[:], in_=w_3[ft])
                            p1 = ps_f1.tile([P, CW], f32, tag="p1")
                            p3 = ps_f3.tile([P, CW], f32, tag="p3")
                            for kt in range(DT):
                                nc.tensor.matmul(p1[:], w1t[:, kt], hn_sb[:, kt, :],
                                                 start=(kt == 0), stop=(kt == DT - 1))
                            for kt in range(DT):
                                nc.tensor.matmul(p3[:], w3t[:, kt], hn_sb[:, kt, :],
                                                 start=(kt == 0), stop=(kt == DT - 1))
                            t1s = st5t.tile([P, CW], f32, tag="t1s")
                            nc.vector.tensor_mul(t1s[:], p1[:], s2rep[:, ch(c)])
                            tsi = st5t.tile([P, CW], CDT, tag="tsi")
                            nc.scalar.activation(out=tsi[:], in_=t1s[:], func=SILU)
                            t2s = st5t.tile([P, CW], f32, tag="t2s")
                            nc.vector.tensor_mul(t2s[:], p3[:], tsi[:])
                            nc.gpsimd.tensor_mul(g_sb[:, ft, :], t2s[:],
                                                 s2rep[:, ch(c)])
                        for ot in range(32):
                            w2t = st5w2.tile([P, FT, P], CDT, tag="w2")
                            nc.sync.dma_start(out=w2t[:], in_=w_2[ot])
                            pt = ps_w2.tile([P, CW], f32, tag="pw2")
                            for ft in range(FT):
                                nc.tensor.matmul(pt[:], w2t[:, ft], g_sb[:, ft, :],
                                                 start=(ft == 0), stop=(ft == FT - 1))
                            og = st5t.tile([P, CW], CDT, tag="og")
                            if ot % 2 == 0:
                                nc.vector.tensor_copy(out=og[:], in_=pt[:])
                            else:
                                nc.scalar.copy(out=og[:], in_=pt[:])
                            nc.sync.dma_start(out=oT_cc[c][P * ot:P * (ot + 1), :],
                                              in_=og[:])
                        nc.gpsimd.collective_compute(
                            "ReduceScatter", ADD, ins=[oT_cc[c][:]],
                            outs=[o_rs[c][:]], replica_groups=RG)

            # ============ stage 6: final residual ============
            with tc.tile_pool(name="st6", bufs=2) as st6:
                for c in range(NCH):
                    for i in range(4):
                        o_sb = st6.tile([P, CW], CDT, tag="osb")
                        nc.gpsimd.dma_start(out=o_sb[:],
                                            in_=o_rs[c][P * i:P * (i + 1), :])
                        out_sb = st6.tile([P, CW], f32, tag="outsb")
                        nc.vector.tensor_add(out_sb[:], hT[i][:, ch(c)], o_sb[:])
                        nc.sync.dma_start(out=outT_s[P * i:P * (i + 1), ch(c)],
                                          in_=out_sb[:])

    nc.compile()
    return nc


def _prep_inputs(x, freqs_cos, freqs_sin, mask, attn_norm_w, wq, wk, wv, wo,
                 ffn_norm_w, w1, w2, w3):
    """Host-side sharding + weight layout. Returns in_maps for 8 cores."""
    f32 = np.float32
    x2 = np.asarray(x, f32)[0]                     # [S, D]
    xT = np.ascontiguousarray(x2.T)                # [D, S]
    anw = np.asarray(attn_norm_w, f32)
    fnw = np.asarray(ffn_norm_w, f32)
    wq = np.asarray(wq, f32) * anw[None, :]
    wk = np.asarray(wk, f32) * anw[None, :]
    wv_e = np.asarray(wv, f32)
    wo = np.asarray(wo, f32)
    w1 = np.asarray(w1, f32) * fnw[None, :]
    w3 = np.asarray(w3, f32) * fnw[None, :]
    w2 = np.asarray(w2, f32)

    perm = np.concatenate([np.arange(0, HD, 2), np.arange(1, HD, 2)])

    cosT = np.ascontiguousarray(np.asarray(freqs_cos, f32).T)   # [64, S]
    sinT = np.ascontiguousarray(np.asarray(freqs_sin, f32).T)
    cos2 = np.concatenate([cosT, cosT], axis=0).astype(NP_CDT)  # [128, S]
    sinsg2 = np.concatenate([-sinT, sinT], axis=0).astype(NP_CDT)
    m = np.asarray(mask, f32)[0, 0]
    dmask = (np.ascontiguousarray(m[:P, :P].T) * f32(math.sqrt(HD))).astype(f32)

    def lhsT_tiles(wt, n_out_tiles, n_k_tiles):
        # wt: [K, Mout] -> [ot, p, kt, j] with [ot,p,kt,j] = wt[128*kt+p, 128*ot+j]
        a = wt.reshape(n_k_tiles, P, n_out_tiles, P)
        return np.ascontiguousarray(a.transpose(2, 1, 0, 3)).astype(NP_CDT)

    in_maps = []
    for r in range(CORES):
        ds = slice(DQ * r, DQ * (r + 1))
        wqT = wq[ds].T.copy()                      # [D, DQ]
        wkT = wk[ds].T.copy()
        for h in range(NHC):
            blk = slice(HD * h, HD * (h + 1))
            wqT[:, blk] = wqT[:, blk][:, perm]
            wkT[:, blk] = wkT[:, blk][:, perm]
        wqk = np.concatenate([lhsT_tiles(wqT, NHC, DT),
                              lhsT_tiles(wkT, NHC, DT)], axis=0)  # [8,P,DT,P]
        wvT = wv_e[ds].T.copy()                    # [D, DQ]
        w_v_l = np.ascontiguousarray(wvT.reshape(DT, P, DQ)).astype(NP_CDT)
        woT = wo[:, ds].T.copy()                   # [DQ, D]
        wo_l = lhsT_tiles(woT, 32, 4)              # [32, P, 4, P]
        wo_l = np.ascontiguousarray(wo_l.transpose(1, 0, 2, 3))  # [P,32,4,P]
        fs = slice(FC * r, FC * (r + 1))
        w1s = np.zeros((FP, D), f32)
        w3s = np.zeros((FP, D), f32)
        w1s[:FC] = w1[fs]
        w3s[:FC] = w3[fs]
        w1_l = lhsT_tiles(np.ascontiguousarray(w1s.T), FT, DT)  # [FT, P, DT, P]
        w3_l = lhsT_tiles(np.ascontiguousarray(w3s.T), FT, DT)
        w2s = np.zeros((FP, D), f32)
        w2s[:FC] = w2[:, fs].T                     # [FP, D] (rows = f)
        w2_l = lhsT_tiles(w2s, 32, FT)             # [32, P, FT, P]

        in_maps.append({
            "xT_s": np.ascontiguousarray(xT[ds]),
            "xT_bf": np.ascontiguousarray(xT[ds]).astype(NP_CDT),
            "w_qk": wqk,
            "w_v": w_v_l,
            "w_o": wo_l,
            "w_1": w1_l,
            "w_3": w3_l,
            "w_2": w2_l,
            "cos2": cos2,
            "sinsg2": sinsg2,
            "dmask": dmask,
        })
    return in_maps


def kernel(x, freqs_cos, freqs_sin, mask, attn_norm_w, wq, wk, wv, wo,
           ffn_norm_w, w1, w2, w3, _trace=False):
    global _COMPILED
    if _COMPILED is None:
        _COMPILED = _build()
    nc = _COMPILED
    in_maps = _prep_inputs(x, freqs_cos, freqs_sin, mask, attn_norm_w,
                           wq, wk, wv, wo, ffn_norm_w, w1, w2, w3)
    res = run_bass_kernel_spmd(nc, in_maps, list(range(CORES)), trace=_trace)
    kernel.last_result = res
    outT = np.concatenate([res.results[r]["outT_s"] for r in range(CORES)],
                          axis=0)                  # [D, S]
    return np.ascontiguousarray(outT.T)[None].astype(np.float32)

